# revision 1
# baseline (speedup 1.0000x reference)
"""Trainium2 Bass kernel for nn_Model2 (5x 4D conv + 2 FC), data-parallel over 8 cores.

Algorithm (per core, batch shard of 32):
  - Activations live in SBUF as [partitions=(channel, d4-position), free=(batch, d1, d2, d3)].
  - Each 4D conv becomes k^3 PSUM-accumulating matmuls: lhsT is a host-built
    block-Toeplitz weight matrix along d4 [(ci,i4) x (co,o4)], one per
    (delta1,delta2,delta3) kernel offset; the rhs for each offset is the SAME
    SBUF tile viewed with a shifted free-dim access pattern (zero data movement).
  - A layer's PSUM output partition layout (co,o4) is exactly the next layer's
    required input layout -> no transposes anywhere.
  - ReLU + bias fused into the PSUM->SBUF evacuation on the scalar engine.
  - FC1 = 64 accumulating matmuls over the (o1,o2,o3) free index; FC2 + sigmoid.
All weight reshaping is done on the host in numpy; weights are embedded in the
NEFF as inline consts. Compute dtype bf16 (fp32 PSUM accumulate).
"""
import numpy as np
import ml_dtypes
from contextlib import ExitStack
from itertools import product

# ---------------- hardcoded problem config ----------------
B = 256
N_CORES = 8
B_SH = B // N_CORES          # 32 per core
B_C = 4                      # batch elements per chunk
N_CHUNKS = B_SH // B_C
CONV_CFG = [(1, 3, 4), (3, 9, 4), (9, 12, 4), (12, 15, 4), (15, 15, 3)]
S_IN = 18                    # input spatial extent (all 4 dims)
FLAT = 15 * 4 ** 4           # 3840
FC1_N = 100

_BF16 = ml_dtypes.bfloat16


class _LayerCfg:
    def __init__(self, ci, co, k, sin):
        self.ci, self.co, self.k, self.sin = ci, co, k, sin
        self.sout = sin - k + 1
        self.K = ci * sin          # contraction rows (ci, i4)
        self.M = co * self.sout    # output rows (co, o4)
        self.n_acc = k ** 3        # (d1,d2,d3) accumulation steps


def _layers():
    ls, s = [], S_IN
    for ci, co, k in CONV_CFG:
        L = _LayerCfg(ci, co, k, s)
        ls.append(L)
        s = L.sout
    return ls


def _toeplitz_weights(w, L):
    """w: [co, ci, k,k,k,k] -> Wt [K, n_acc, M] with
    Wt[(ci,i4), (d1,d2,d3), (co,o4)] = w[co,ci,d1,d2,d3,i4-o4]."""
    k, ci_n, co_n, sin, sout = L.k, L.ci, L.co, L.sin, L.sout
    Wt = np.zeros((L.K, L.n_acc, L.M), np.float32)
    wv = np.asarray(w, np.float32)
    ci_idx = np.arange(ci_n)
    co_idx = np.arange(co_n)
    for d1, d2, d3 in product(range(k), repeat=3):
        di = (d1 * k + d2) * k + d3
        for d4 in range(k):
            for o4 in range(sout):
                i4 = o4 + d4
                blk = wv[:, :, d1, d2, d3, d4]           # [co, ci]
                Wt[(ci_idx * sin + i4)[:, None], di,
                   (co_idx * sout + o4)[None, :]] = blk.T
    return Wt


def _host_prep(inputs):
    """Build per-core x shards and all device constants."""
    x = np.asarray(inputs["x"], np.float32)      # [B,1,18,18,18,18]
    Ls = _layers()
    consts = {}
    for i, L in enumerate(Ls, start=1):
        Wt = _toeplitz_weights(inputs[f"w{i}"], L)
        consts[f"cw{i}"] = np.ascontiguousarray(Wt).astype(_BF16)
        bias = np.repeat(np.asarray(inputs[f"b{i}"], np.float32), L.sout)
        consts[f"cb{i}"] = np.ascontiguousarray(bias.reshape(L.M, 1))
    # FC1: rows (co,o1,o2,o3,o4) -> [(co,o4), (o1o2o3), 100]
    f1 = np.asarray(inputs["fc1_w"], np.float32).reshape(15, 4, 4, 4, 4, FC1_N)
    f1 = f1.transpose(0, 4, 1, 2, 3, 5).reshape(60, 64, FC1_N)
    consts["cfc1w"] = np.ascontiguousarray(f1).astype(_BF16)
    consts["cfc1b"] = np.ascontiguousarray(np.asarray(inputs["fc1_b"], np.float32).reshape(FC1_N, 1))
    consts["cfc2w"] = np.ascontiguousarray(np.asarray(inputs["fc2_w"], np.float32)).astype(_BF16)
    fc2b = float(np.asarray(inputs["fc2_b"], np.float32).reshape(-1)[0])

    # x per core: [B_SH, i4, s1, s2, s3] bf16
    shards = []
    for c in range(N_CORES):
        xs = x[c * B_SH:(c + 1) * B_SH, 0]                 # [32,18,18,18,18]
        xs = np.ascontiguousarray(xs.transpose(0, 4, 1, 2, 3)).astype(_BF16)
        shards.append(xs)
    return Ls, consts, fc2b, shards


def _build_module(Ls, consts, fc2b):
    import concourse.bass as bass
    import concourse.tile as tile
    import concourse.mybir as mybir

    dt = mybir.dt
    AF = mybir.ActivationFunctionType
    nc = bass.Bass("TRN2", target_bir_lowering=False, debug=False,
                   enable_asserts=False, num_devices=N_CORES)

    x_d = nc.dram_tensor("x", [B_SH, S_IN, S_IN ** 3], dt.bfloat16, kind="ExternalInput").ap()
    out_d = nc.dram_tensor("out", [1, B_SH], dt.float32, kind="ExternalOutput").ap()
    cw_d = [nc.inline_tensor(consts[f"cw{i}"], name=f"cw{i}") for i in range(1, 6)]
    cb_d = [nc.inline_tensor(consts[f"cb{i}"], name=f"cb{i}") for i in range(1, 6)]
    fc1w_d = nc.inline_tensor(consts["cfc1w"], name="cfc1w")
    fc1b_d = nc.inline_tensor(consts["cfc1b"], name="cfc1b")
    fc2w_d = nc.inline_tensor(consts["cfc2w"], name="cfc2w")

    with tile.TileContext(nc) as tc:
        with ExitStack() as ctx:
            wpool = ctx.enter_context(tc.tile_pool(name="wts", bufs=1))
            xpool = ctx.enter_context(tc.tile_pool(name="xin", bufs=3))
            apool = ctx.enter_context(tc.tile_pool(name="acts", bufs=1))
            hpool = ctx.enter_context(tc.tile_pool(name="hbuf", bufs=1))
            opool = ctx.enter_context(tc.tile_pool(name="outb", bufs=1))
            pspool = ctx.enter_context(tc.tile_pool(name="ps", bufs=4, space="PSUM"))
            fcps = ctx.enter_context(tc.tile_pool(name="fcps", bufs=1, space="PSUM"))

            # ---- load constants ----
            w_sb, b_sb = [], []
            for i, L in enumerate(Ls):
                wt = wpool.tile([L.K, L.n_acc, L.M], dt.bfloat16, tag=f"w{i}")
                nc.sync.dma_start(wt[:], cw_d[i].ap())
                w_sb.append(wt)
                bt = wpool.tile([L.M, 1], dt.float32, tag=f"b{i}")
                nc.sync.dma_start(bt[:], cb_d[i].ap())
                b_sb.append(bt)
            fc1w = wpool.tile([60, 64, FC1_N], dt.bfloat16, tag="fc1w")
            nc.sync.dma_start(fc1w[:], fc1w_d.ap())
            fc1b = wpool.tile([FC1_N, 1], dt.float32, tag="fc1b")
            nc.sync.dma_start(fc1b[:], fc1b_d.ap())
            fc2w = wpool.tile([FC1_N, 1], dt.bfloat16, tag="fc2w")
            nc.sync.dma_start(fc2w[:], fc2w_d.ap())

            h_all = hpool.tile([60, B_SH, 4, 4, 4], dt.bfloat16)

            # ---- conv stack, batch-chunked ----
            for chunk in range(N_CHUNKS):
                # per-batch-element x tiles
                xts = []
                for bi in range(B_C):
                    b_glob = chunk * B_C + bi
                    xt = xpool.tile([S_IN, S_IN, S_IN, S_IN], dt.bfloat16, tag="xt")
                    nc.sync.dma_start(
                        xt[:], x_d[b_glob].rearrange("p (a b c) -> p a b c", a=S_IN, b=S_IN))
                    xts.append(xt)

                cur = None  # act tile of previous layer [M, B_C, O1, O2, O3]
                for li, L in enumerate(Ls):
                    O1 = O2 = O3 = L.sout
                    S1 = S2 = S3 = L.sin
                    k = L.k
                    # output tile (except L5 writes into h_all)
                    last = li == len(Ls) - 1
                    if not last:
                        nxt = apool.tile([L.M, B_C, O1, O2, O3], dt.bfloat16, tag=f"a{li}")
                    # o1 tiling to fit psum bank (<=512 fp32)
                    o1_t = max(1, 512 // (O2 * O3))
                    for bi in range(B_C):
                        src = xts[bi] if li == 0 else cur
                        o1a = 0
                        while o1a < O1:
                            o1b = min(o1a + o1_t, O1)
                            no1 = o1b - o1a
                            ps = pspool.tile([L.M, no1, O2, O3], dt.float32, tag="ps")
                            di = 0
                            for d1, d2, d3 in product(range(k), repeat=3):
                                if li == 0:
                                    rhs = src[0:L.K, d1 + o1a:d1 + o1b, d2:d2 + O2, d3:d3 + O3]
                                else:
                                    rhs = src[0:L.K, bi, d1 + o1a:d1 + o1b, d2:d2 + O2, d3:d3 + O3]
                                nc.tensor.matmul(ps[:], w_sb[li][:, di, :], rhs,
                                                 start=(di == 0), stop=(di == L.n_acc - 1))
                                di += 1
                            if last:
                                dst = h_all[0:L.M, chunk * B_C + bi, :, :, :]
                            else:
                                dst = nxt[0:L.M, bi, o1a:o1b, :, :]
                            nc.scalar.activation(dst, ps[:], AF.Relu, bias=b_sb[li][:])
                            o1a = o1b
                    if not last:
                        cur = nxt

            # ---- FC head ----
            ps1 = fcps.tile([FC1_N, B_SH], dt.float32, tag="psfc")
            for f in range(64):
                x0, y0, z0 = f // 16, (f // 4) % 4, f % 4
                nc.tensor.matmul(ps1[:], fc1w[:, f, :], h_all[:, :, x0, y0, z0],
                                 start=(f == 0), stop=(f == 63))
            r1 = opool.tile([FC1_N, B_SH], dt.bfloat16, tag="r1")
            nc.scalar.activation(r1[:], ps1[:], AF.Relu, bias=fc1b[:])
            ps2 = fcps.tile([1, B_SH], dt.float32, tag="psfc2")
            nc.tensor.matmul(ps2[:], fc2w[:], r1[:], start=True, stop=True)
            ob = opool.tile([1, B_SH], dt.float32, tag="ob")
            nc.scalar.activation(ob[:], ps2[:], AF.Sigmoid, bias=fc2b)
            nc.sync.dma_start(out_d, ob[:])

    _split_excess_waits(nc)
    return nc


def _split_excess_waits(nc, max_waits=1):
    """This toolchain's walrus accepts very few sem-waits per instruction
    (1 for most opcodes). Tile emits instructions whose sync_info carries one
    wait per producer proc. Hoist excess waits onto injected wait-only
    InstEventSemaphore carriers placed immediately before the instruction on
    the same engine (semantically identical: the engine blocks at the same
    program point)."""
    import concourse.mybir as mybir
    f = nc.m.functions[0]
    ctr = 0
    for blk in f.blocks:
        il = blk.instructions
        i = 0
        while i < len(il):
            inst = il[i]
            si = inst.sync_info
            ty = type(inst).__name__
            lim = 2 if ty == "InstEventSemaphore" else max_waits
            if si is not None and si.on_wait and len(si.on_wait) > lim:
                waits = list(si.on_wait)
                si.on_wait = waits[:lim]
                for w in waits[lim:]:
                    ev = mybir.InstEventSemaphore(name=f"wsplit_{ctr}", ins=[], outs=[])
                    ctr += 1
                    ev.engine = inst.engine
                    ev.sync_info = mybir.SyncInfo(on_wait=[w], on_update=[])
                    il.insert(i, ev)
                    i += 1
            i += 1
    return ctr


def kernel(**inputs) -> np.ndarray:
    import concourse.bass_utils as bass_utils
    Ls, consts, fc2b, shards = _host_prep(inputs)
    nc = _build_module(Ls, consts, fc2b)
    in_maps = [{"x": s.reshape(B_SH, S_IN, S_IN ** 3)} for s in shards]
    r = bass_utils.run_bass_kernel_spmd(nc, in_maps, core_ids=list(range(N_CORES)))
    outs = [r.results[c]["out"].reshape(B_SH, 1) for c in range(N_CORES)]
    return np.concatenate(outs, axis=0).astype(np.float32)


if __name__ == "__main__":
    rng = np.random.default_rng(0)
    ins = {"x": rng.normal(size=(B, 1, 18, 18, 18, 18)).astype(np.float32)}
    s = 18
    for i, (ci, co, k) in enumerate(CONV_CFG, start=1):
        ins[f"w{i}"] = (rng.normal(size=(co, ci, k, k, k, k)) / np.sqrt(ci * k ** 4)).astype(np.float32)
        ins[f"b{i}"] = np.zeros((co,), np.float32)
    ins["fc1_w"] = (rng.normal(size=(FLAT, FC1_N)) / np.sqrt(FLAT)).astype(np.float32)
    ins["fc1_b"] = np.zeros((FC1_N,), np.float32)
    ins["fc2_w"] = (rng.normal(size=(FC1_N, 1)) / 10.0).astype(np.float32)
    ins["fc2_b"] = np.zeros((1,), np.float32)
    out = kernel(**ins)
    print("out", out.shape, out[:4, 0])



# revision 2
# speedup vs baseline: 1.5272x; 1.5272x over previous
"""Trainium2 Bass kernel for nn_Model2 (5x 4D conv + 2 FC), data-parallel over 8 cores.

V2: 2D-Toeplitz formulation for L1/L2 to raise PE-array utilization.

Layer mapping (per core, batch shard of 32, chunks of 4):
  L1 (1->3, k=4, 18->15):  K=(i3 win, i4 win), M=(o3 blk, o4 blk, co), accum (d1,d2)=16.
     Output tiles (T3,T4)=(5,8)/(5,7): 6 psum tiles/elem vs 216k cols baseline.
  L2 (3->9, k=4, 15->12):  K=(i3 win6, i4 win7, ci)=126, M=(o3 3, o4 4, co)=108,
     12 exact tiles, accum 16.  Input windows materialized by partition-scatter
     DMAs from L1 output (channel-innermost layouts keep every scatter a
     contiguous partition range).
  L3/L4/L5: 1D-Toeplitz (K=(i4,ci), M=(o4,co), accum k^3) — partition-aligned
     chaining, no scatters.
  FC1 = 64 accumulating matmuls over (o3,o1,o2); FC2 + sigmoid.
All weights host-reshaped into lhsT consts; compute bf16, fp32 PSUM.
"""
import numpy as np
import ml_dtypes
from contextlib import ExitStack
from itertools import product

# ---------------- hardcoded problem config ----------------
B = 256
N_CORES = 8
B_SH = B // N_CORES          # 32 per core
B_C = 4                      # batch elements per chunk
N_CHUNKS_FULL = B_SH // B_C
CONV_CFG = [(1, 3, 4), (3, 9, 4), (9, 12, 4), (12, 15, 4), (15, 15, 3)]
S_IN = 18
FLAT = 15 * 4 ** 4           # 3840
FC1_N = 100

_BF16 = ml_dtypes.bfloat16

# L1 output tiling: (o3a, T3, o4a, T4); sout=15
L1_TILES = [(o3a, 5, o4a, t4) for o3a in (0, 5, 10) for (o4a, t4) in ((0, 8), (8, 7))]
L1_KMAX = 88                                     # max (T3+3)*(T4+3)
# L2 output tiling: (3,4) exact; windows into L1 output
L2_T3, L2_T4 = 3, 4
L2_W3, L2_W4 = L2_T3 + 3, L2_T4 + 3              # 6, 7
L2_K = L2_W3 * L2_W4 * 3                         # 126
L2_M = L2_T3 * L2_T4 * 9                         # 108


def _w1_lhsT(w1, T3, T4):
    """[K=(i3r,i4r), 16, M=(o3r,o4r,co)] band lhsT for an L1 tile shape."""
    W3, W4 = T3 + 3, T4 + 3
    K, M = W3 * W4, T3 * T4 * 3
    out = np.zeros((K, 16, M), np.float32)
    w = np.asarray(w1, np.float32)               # [3,1,4,4,4,4]
    for o3r, o4r, co in product(range(T3), range(T4), range(3)):
        m = (o3r * T4 + o4r) * 3 + co
        for d3, d4 in product(range(4), range(4)):
            out[(o3r + d3) * W4 + (o4r + d4), :, m] = \
                w[co, 0, :, :, d3, d4].reshape(16)
    return out


def _w2_lhsT(w2):
    """[K=(i3r,i4r,ci)=126, 16, M=(o3r,o4r,co)=108]."""
    out = np.zeros((L2_K, 16, L2_M), np.float32)
    w = np.asarray(w2, np.float32)               # [9,3,4,4,4,4]
    for o3r, o4r, co in product(range(L2_T3), range(L2_T4), range(9)):
        m = (o3r * L2_T4 + o4r) * 9 + co
        for d3, d4, ci in product(range(4), range(4), range(3)):
            out[((o3r + d3) * L2_W4 + (o4r + d4)) * 3 + ci, :, m] = \
                w[co, ci, :, :, d3, d4].reshape(16)
    return out


def _w_1d_lhsT(w, ci_n, co_n, k, sin, sout):
    """[K=(i4,ci), k^3, M=(o4,co)] banded-i4 lhsT (channel innermost)."""
    K, M = sin * ci_n, sout * co_n
    out = np.zeros((K, k ** 3, M), np.float32)
    w = np.asarray(w, np.float32)
    for o4, co in product(range(sout), range(co_n)):
        m = o4 * co_n + co
        for d4, ci in product(range(k), range(ci_n)):
            out[(o4 + d4) * ci_n + ci, :, m] = w[co, ci, :, :, :, d4].reshape(k ** 3)
    return out


def _host_prep(inputs):
    consts = {}
    # ---- L1 lhsT (2 tile shapes) + biases ----
    consts["w1f"] = _w1_lhsT(inputs["w1"], 5, 8).astype(_BF16)      # [88,16,120]
    consts["w1e"] = _w1_lhsT(inputs["w1"], 5, 7).astype(_BF16)      # [80,16,105]
    b1 = np.asarray(inputs["b1"], np.float32)
    consts["b1f"] = np.tile(b1, 40).reshape(120, 1).copy()
    consts["b1e"] = np.tile(b1, 35).reshape(105, 1).copy()
    # ---- L2 ----
    consts["w2t"] = _w2_lhsT(inputs["w2"]).astype(_BF16)            # [126,16,108]
    consts["b2t"] = np.tile(np.asarray(inputs["b2"], np.float32), 12).reshape(L2_M, 1).copy()
    # ---- L3..L5 ----
    for i, (ci, co, k) in enumerate(CONV_CFG[2:], start=3):
        sin = {3: 12, 4: 9, 5: 6}[i]
        sout = sin - k + 1
        consts[f"w{i}t"] = _w_1d_lhsT(inputs[f"w{i}"], ci, co, k, sin, sout).astype(_BF16)
        consts[f"b{i}t"] = np.tile(np.asarray(inputs[f"b{i}"], np.float32),
                                   sout).reshape(sout * co, 1).copy()
    # ---- FC ----
    f1 = np.asarray(inputs["fc1_w"], np.float32).reshape(15, 4, 4, 4, 4, FC1_N)
    # rows (o4, co), f index (o3, o1, o2)
    f1 = f1.transpose(4, 0, 3, 1, 2, 5).reshape(60, 64, FC1_N)
    consts["fc1w"] = np.ascontiguousarray(f1).astype(_BF16)
    consts["fc1b"] = np.ascontiguousarray(np.asarray(inputs["fc1_b"], np.float32).reshape(FC1_N, 1))
    consts["fc2w"] = np.ascontiguousarray(np.asarray(inputs["fc2_w"], np.float32)).astype(_BF16)
    fc2b = float(np.asarray(inputs["fc2_b"], np.float32).reshape(-1)[0])

    # ---- x shards: windowed layout [6, 88, B_SH, 18, 18] per core ----
    x = np.asarray(inputs["x"], np.float32)
    shards = []
    for c in range(N_CORES):
        xs = x[c * B_SH:(c + 1) * B_SH, 0]       # [B_SH,18,18,18,18] (i1,i2,i3,i4)
        xw = np.zeros((6, L1_KMAX, B_SH, S_IN, S_IN), np.float32)
        for t, (o3a, T3, o4a, T4) in enumerate(L1_TILES):
            W3, W4 = T3 + 3, T4 + 3
            win = xs[:, :, :, o3a:o3a + W3, o4a:o4a + W4]  # [B,18,18,W3,W4]
            xw[t, :W3 * W4] = win.transpose(3, 4, 0, 1, 2).reshape(W3 * W4, B_SH, S_IN, S_IN)
        shards.append(np.ascontiguousarray(xw).astype(_BF16))
    return consts, fc2b, shards


def _scatter_plan():
    """Per L1-source-tile list of (window, dst_part, src_part, n_part) frags,
    plus per-window readiness (last source tile index)."""
    by_src = {t: [] for t in range(6)}
    ready = {}
    for u, v in product(range(4), range(3)):
        last = 0
        for i3r in range(L2_W3):
            o3 = 3 * u + i3r
            s, o3r = divmod(o3, 5)
            for g0, g1 in ((4 * v, min(4 * v + 7, 8)), (max(4 * v, 8), 4 * v + 7)):
                if g0 >= g1:
                    continue
                tt = 0 if g0 < 8 else 1
                o4a, T4 = (0, 8) if tt == 0 else (8, 7)
                st = s * 2 + tt
                sp = (o3r * T4 + (g0 - o4a)) * 3
                dp = (i3r * L2_W4 + (g0 - 4 * v)) * 3
                by_src[st].append(((u, v), dp, sp, (g1 - g0) * 3))
                last = max(last, st)
        ready[(u, v)] = last
    order = sorted(ready, key=lambda w: (ready[w], w))
    return by_src, order


def _build_module(consts, fc2b, n_chunks=N_CHUNKS_FULL, split_waits=True):
    import concourse.bass as bass
    import concourse.tile as tile
    import concourse.mybir as mybir

    b_sh = n_chunks * B_C
    dt = mybir.dt
    AF = mybir.ActivationFunctionType
    nc = bass.Bass("TRN2", target_bir_lowering=False, debug=False,
                   enable_asserts=False, num_devices=N_CORES)

    x_d = nc.dram_tensor("x", [6, L1_KMAX, b_sh, S_IN, S_IN], dt.bfloat16,
                         kind="ExternalInput").ap()
    out_d = nc.dram_tensor("out", [1, b_sh], dt.float32, kind="ExternalOutput").ap()
    cd = {k: nc.inline_tensor(v, name=k) for k, v in consts.items()}

    with tile.TileContext(nc) as tc:
        with ExitStack() as ctx:
            wpool = ctx.enter_context(tc.tile_pool(name="wts", bufs=1))
            xpool = ctx.enter_context(tc.tile_pool(name="xin", bufs=2))
            apool = ctx.enter_context(tc.tile_pool(name="acts", bufs=1))
            rpool = ctx.enter_context(tc.tile_pool(name="rbuf", bufs=2))
            opool = ctx.enter_context(tc.tile_pool(name="outb", bufs=1))
            pspool = ctx.enter_context(tc.tile_pool(name="ps", bufs=4, space="PSUM"))
            fcps = ctx.enter_context(tc.tile_pool(name="fcps", bufs=1, space="PSUM"))

            # ---- load constants to SBUF ----
            sb = {}
            for k, v in consts.items():
                t = wpool.tile(list(v.shape),
                               dt.bfloat16 if v.dtype == _BF16 else dt.float32, tag=k)
                nc.sync.dma_start(t[:], cd[k].ap())
                sb[k] = t

            h_all = apool.tile([60, b_sh, 64], dt.bfloat16)   # (o4,co) x (b, o3o1o2)
            by_src, worder = _scatter_plan()
            dmaq = [nc.sync, nc.gpsimd]          # alternate HWDGE / SWDGE

            for chunk in range(n_chunks):
                b0 = chunk * B_C
                qi = 0
                # ---- input DMA (windowed layout, one per L1 tile) ----
                xw = []
                for t, (o3a, T3, o4a, T4) in enumerate(L1_TILES):
                    Kt = (T3 + 3) * (T4 + 3)
                    xt = xpool.tile([L1_KMAX, B_C, S_IN, S_IN], dt.bfloat16, tag=f"xw{t}")
                    nc.sync.dma_start(xt[0:Kt], x_d[t, 0:Kt, b0:b0 + B_C])
                    xw.append(xt)

                # window tiles allocated up front so scatters can land early
                a2w = {w: apool.tile([L2_K, B_C, 15, 15], dt.bfloat16,
                                     tag=f"a2w{w[0]}_{w[1]}",
                                     name=f"a2w{w[0]}_{w[1]}")
                       for w in product(range(4), range(3))}

                # ---- L1 (+ scatter frags as soon as their source is done) ----
                for t, (o3a, T3, o4a, T4) in enumerate(L1_TILES):
                    Kt, Mt = (T3 + 3) * (T4 + 3), T3 * T4 * 3
                    wt = sb["w1f"] if T4 == 8 else sb["w1e"]
                    bt = sb["b1f"] if T4 == 8 else sb["b1e"]
                    rt = rpool.tile([Mt, B_C, 15, 15], dt.bfloat16, tag=f"r1_{t}")
                    for h in range(2):
                        ps = pspool.tile([Mt, 2, 15, 15], dt.float32, tag="ps")
                        for di, (d1, d2) in enumerate(product(range(4), range(4))):
                            nc.tensor.matmul(
                                ps[:], wt[0:Kt, di, :],
                                xw[t][0:Kt, 2 * h:2 * h + 2, d1:d1 + 15, d2:d2 + 15],
                                start=(di == 0), stop=(di == 15))
                        nc.scalar.activation(rt[:, 2 * h:2 * h + 2], ps[:],
                                             AF.Relu, bias=bt[:])
                    for w, dp, sp, n in by_src[t]:
                        dmaq[qi % 2].dma_start(a2w[w][dp:dp + n], rt[sp:sp + n])
                        qi += 1

                # ---- L2 (windows in scatter-readiness order) ----
                a3 = apool.tile([108, B_C, 12, 12, 12], dt.bfloat16, tag="a3")
                for u, v in worder:
                    rt = rpool.tile([L2_M, B_C, 12, 12], dt.bfloat16, tag="r2", bufs=3)
                    for h in range(2):
                        ps = pspool.tile([L2_M, 2, 12, 12], dt.float32, tag="ps")
                        for di, (d1, d2) in enumerate(product(range(4), range(4))):
                            nc.tensor.matmul(
                                ps[:], sb["w2t"][:, di, :],
                                a2w[(u, v)][:, 2 * h:2 * h + 2, d1:d1 + 12, d2:d2 + 12],
                                start=(di == 0), stop=(di == 15))
                        nc.scalar.activation(rt[:, 2 * h:2 * h + 2], ps[:],
                                             AF.Relu, bias=sb["b2t"][:])
                    # scatter -> a3 [(i4,ci), (b, i3, i1, i2)]
                    for o3r in range(3):
                        dmaq[qi % 2].dma_start(a3[36 * v:36 * v + 36, :, 3 * u + o3r],
                                               rt[36 * o3r:36 * o3r + 36])
                        qi += 1

                # ---- L3 ----
                a4 = apool.tile([108, B_C, 9, 9, 9], dt.bfloat16, tag="a4")
                for bi in range(B_C):
                    for c0, cl in ((0, 5), (5, 4)):
                        ps = pspool.tile([108, cl, 9, 9], dt.float32, tag="ps")
                        for di, (d1, d2, d3) in enumerate(product(range(4), repeat=3)):
                            nc.tensor.matmul(
                                ps[:], sb["w3t"][:, di, :],
                                a3[:, bi, d3 + c0:d3 + c0 + cl, d1:d1 + 9, d2:d2 + 9],
                                start=(di == 0), stop=(di == 63))
                        nc.scalar.activation(a4[:, bi, c0:c0 + cl], ps[:],
                                             AF.Relu, bias=sb["b3t"][:])

                # ---- L4 ----
                a5 = apool.tile([90, B_C, 6, 6, 6], dt.bfloat16, tag="a5")
                for bi in range(B_C):
                    ps = pspool.tile([90, 6, 6, 6], dt.float32, tag="ps")
                    for di, (d1, d2, d3) in enumerate(product(range(4), repeat=3)):
                        nc.tensor.matmul(
                            ps[:], sb["w4t"][:, di, :],
                            a4[:, bi, d3:d3 + 6, d1:d1 + 6, d2:d2 + 6],
                            start=(di == 0), stop=(di == 63))
                    nc.scalar.activation(a5[:, bi], ps[:], AF.Relu, bias=sb["b4t"][:])

                # ---- L5 ----
                ps5 = pspool.tile([60, B_C, 4, 4, 4], dt.float32, tag="ps")
                for bi in range(B_C):
                    for di, (d1, d2, d3) in enumerate(product(range(3), repeat=3)):
                        nc.tensor.matmul(
                            ps5[:, bi], sb["w5t"][:, di, :],
                            a5[:, bi, d3:d3 + 4, d1:d1 + 4, d2:d2 + 4],
                            start=(di == 0), stop=(di == 26))
                nc.scalar.activation(
                    h_all[:, b0:b0 + B_C],
                    ps5[:].rearrange("p b x y z -> p b (x y z)"),
                    AF.Relu, bias=sb["b5t"][:])

            # ---- FC head ----
            ps1 = fcps.tile([FC1_N, b_sh], dt.float32, tag="psfc")
            for f in range(64):
                nc.tensor.matmul(ps1[:], sb["fc1w"][:, f, :], h_all[:, :, f],
                                 start=(f == 0), stop=(f == 63))
            r1t = opool.tile([FC1_N, b_sh], dt.bfloat16, tag="rfc")
            nc.scalar.activation(r1t[:], ps1[:], AF.Relu, bias=sb["fc1b"][:])
            ps2 = fcps.tile([1, b_sh], dt.float32, tag="psfc2")
            nc.tensor.matmul(ps2[:], sb["fc2w"][:], r1t[:], start=True, stop=True)
            ob = opool.tile([1, b_sh], dt.float32, tag="ob")
            nc.scalar.activation(ob[:], ps2[:], AF.Sigmoid, bias=fc2b)
            nc.sync.dma_start(out_d, ob[:])

    if split_waits:
        _split_excess_waits(nc)
    return nc


def _split_excess_waits(nc, max_waits=1):
    """Hoist excess sem-waits onto injected wait-only InstEventSemaphore
    carriers immediately before the instruction on the same engine."""
    import concourse.mybir as mybir
    f = nc.m.functions[0]
    ctr = 0
    for blk in f.blocks:
        il = blk.instructions
        i = 0
        while i < len(il):
            inst = il[i]
            si = inst.sync_info
            ty = type(inst).__name__
            lim = 2 if ty == "InstEventSemaphore" else max_waits
            if si is not None and si.on_wait and len(si.on_wait) > lim:
                waits = list(si.on_wait)
                si.on_wait = waits[:lim]
                for w in waits[lim:]:
                    ev = mybir.InstEventSemaphore(name=f"wsplit_{ctr}", ins=[], outs=[])
                    ctr += 1
                    ev.engine = inst.engine
                    ev.sync_info = mybir.SyncInfo(on_wait=[w], on_update=[])
                    il.insert(i, ev)
                    i += 1
            i += 1
    return ctr


def kernel(**inputs) -> np.ndarray:
    import concourse.bass_utils as bass_utils
    consts, fc2b, shards = _host_prep(inputs)
    nc = _build_module(consts, fc2b)
    in_maps = [{"x": s} for s in shards]
    r = bass_utils.run_bass_kernel_spmd(nc, in_maps, core_ids=list(range(N_CORES)))
    outs = [r.results[c]["out"].reshape(B_SH, 1) for c in range(N_CORES)]
    return np.concatenate(outs, axis=0).astype(np.float32)


# revision 3
# speedup vs baseline: 1.5418x; 1.0095x over previous
"""Trainium2 Bass kernel for nn_Model2 (5x 4D conv + 2 FC), data-parallel over 8 cores.

V2: 2D-Toeplitz formulation for L1/L2 to raise PE-array utilization.

Layer mapping (per core, batch shard of 32, chunks of 4):
  L1 (1->3, k=4, 18->15):  K=(i3 win, i4 win), M=(o3 blk, o4 blk, co), accum (d1,d2)=16.
     Output tiles (T3,T4)=(5,8)/(5,7): 6 psum tiles/elem vs 216k cols baseline.
  L2 (3->9, k=4, 15->12):  K=(i3 win6, i4 win7, ci)=126, M=(o3 3, o4 4, co)=108,
     12 exact tiles, accum 16.  Input windows materialized by partition-scatter
     DMAs from L1 output (channel-innermost layouts keep every scatter a
     contiguous partition range).
  L3/L4/L5: 1D-Toeplitz (K=(i4,ci), M=(o4,co), accum k^3) — partition-aligned
     chaining, no scatters.
  FC1 = 64 accumulating matmuls over (o3,o1,o2); FC2 + sigmoid.
All weights host-reshaped into lhsT consts; compute bf16, fp32 PSUM.
"""
import numpy as np
import ml_dtypes
from contextlib import ExitStack
from itertools import product

# ---------------- hardcoded problem config ----------------
B = 256
N_CORES = 8
B_SH = B // N_CORES          # 32 per core
B_C = 4                      # batch elements per chunk
N_CHUNKS_FULL = B_SH // B_C
CONV_CFG = [(1, 3, 4), (3, 9, 4), (9, 12, 4), (12, 15, 4), (15, 15, 3)]
S_IN = 18
FLAT = 15 * 4 ** 4           # 3840
FC1_N = 100

_BF16 = ml_dtypes.bfloat16

# L1 output tiling: (o3a, T3, o4a, T4); sout=15
L1_TILES = [(o3a, 5, o4a, t4) for o3a in (0, 5, 10) for (o4a, t4) in ((0, 8), (8, 7))]
L1_KMAX = 88                                     # max (T3+3)*(T4+3)
# L2 output tiling: (3,4) exact; windows into L1 output
L2_T3, L2_T4 = 3, 4
L2_W3, L2_W4 = L2_T3 + 3, L2_T4 + 3              # 6, 7
L2_K = L2_W3 * L2_W4 * 3                         # 126
L2_M = L2_T3 * L2_T4 * 9                         # 108


def _w1_lhsT(w1, T3, T4):
    """[K=(i3r,i4r), 16, M=(o3r,o4r,co)] band lhsT for an L1 tile shape."""
    W3, W4 = T3 + 3, T4 + 3
    K, M = W3 * W4, T3 * T4 * 3
    out = np.zeros((K, 16, M), np.float32)
    w = np.asarray(w1, np.float32)               # [3,1,4,4,4,4]
    for o3r, o4r, co in product(range(T3), range(T4), range(3)):
        m = (o3r * T4 + o4r) * 3 + co
        for d3, d4 in product(range(4), range(4)):
            out[(o3r + d3) * W4 + (o4r + d4), :, m] = \
                w[co, 0, :, :, d3, d4].reshape(16)
    return out


def _w2_lhsT(w2):
    """[K=(i3r,i4r,ci)=126, 16, M=(o3r,o4r,co)=108]."""
    out = np.zeros((L2_K, 16, L2_M), np.float32)
    w = np.asarray(w2, np.float32)               # [9,3,4,4,4,4]
    for o3r, o4r, co in product(range(L2_T3), range(L2_T4), range(9)):
        m = (o3r * L2_T4 + o4r) * 9 + co
        for d3, d4, ci in product(range(4), range(4), range(3)):
            out[((o3r + d3) * L2_W4 + (o4r + d4)) * 3 + ci, :, m] = \
                w[co, ci, :, :, d3, d4].reshape(16)
    return out


def _w_1d_lhsT(w, ci_n, co_n, k, sin, sout):
    """[K=(i4,ci), k^3, M=(o4,co)] banded-i4 lhsT (channel innermost)."""
    K, M = sin * ci_n, sout * co_n
    out = np.zeros((K, k ** 3, M), np.float32)
    w = np.asarray(w, np.float32)
    for o4, co in product(range(sout), range(co_n)):
        m = o4 * co_n + co
        for d4, ci in product(range(k), range(ci_n)):
            out[(o4 + d4) * ci_n + ci, :, m] = w[co, ci, :, :, :, d4].reshape(k ** 3)
    return out


_F8 = ml_dtypes.float8_e4m3
W3_SCALE = 16.0


def _host_prep(inputs):
    consts = {}
    # ---- L1 lhsT (2 tile shapes) + biases ----
    consts["w1f"] = _w1_lhsT(inputs["w1"], 5, 8).astype(_BF16)      # [88,16,120]
    consts["w1e"] = _w1_lhsT(inputs["w1"], 5, 7).astype(_BF16)      # [80,16,105]
    b1 = np.asarray(inputs["b1"], np.float32)
    consts["b1f"] = np.tile(b1, 40).reshape(120, 1).copy()
    consts["b1e"] = np.tile(b1, 35).reshape(105, 1).copy()
    # ---- L2 ----
    consts["w2t"] = _w2_lhsT(inputs["w2"]).astype(_BF16)            # [126,16,108]
    consts["b2t"] = np.tile(np.asarray(inputs["b2"], np.float32), 12).reshape(L2_M, 1).copy()
    # ---- L3..L5 ----
    for i, (ci, co, k) in enumerate(CONV_CFG[2:], start=3):
        sin = {3: 12, 4: 9, 5: 6}[i]
        sout = sin - k + 1
        wt = _w_1d_lhsT(inputs[f"w{i}"], ci, co, k, sin, sout)
        if i == 3:
            # fp8 DoubleRow: K=(i4,ci)=108 split into i4-halves of 54; slabs
            # = the halves; M zero-padded 108->128 (dual-fp8 ISA rule)
            wt = (wt.reshape(2, 6, 9, 64, 108).transpose(1, 2, 3, 0, 4)
                  .reshape(54, 64, 2, 108))
            wt = np.concatenate([wt, np.zeros((54, 64, 2, 20), np.float32)], axis=3)
            consts["w3t"] = (np.ascontiguousarray(wt) * W3_SCALE).astype(_F8)
        else:
            consts[f"w{i}t"] = wt.astype(_BF16)
        consts[f"b{i}t"] = np.tile(np.asarray(inputs[f"b{i}"], np.float32),
                                   sout).reshape(sout * co, 1).copy()
    consts["a3pad"] = np.zeros((54, B_C, 2, 64), _F8)  # zero fill for a3 tail
    # ---- FC ----
    f1 = np.asarray(inputs["fc1_w"], np.float32).reshape(15, 4, 4, 4, 4, FC1_N)
    # rows (o4, co), f index (o3, o1, o2)
    f1 = f1.transpose(4, 0, 3, 1, 2, 5).reshape(60, 64, FC1_N)
    consts["fc1w"] = np.ascontiguousarray(f1).astype(_BF16)
    consts["fc1b"] = np.ascontiguousarray(np.asarray(inputs["fc1_b"], np.float32).reshape(FC1_N, 1))
    consts["fc2w"] = np.ascontiguousarray(np.asarray(inputs["fc2_w"], np.float32)).astype(_BF16)
    fc2b = float(np.asarray(inputs["fc2_b"], np.float32).reshape(-1)[0])

    # ---- x shards: windowed layout [6, 88, B_SH, 18, 18] per core ----
    x = np.asarray(inputs["x"], np.float32)
    shards = []
    for c in range(N_CORES):
        xs = x[c * B_SH:(c + 1) * B_SH, 0]       # [B_SH,18,18,18,18] (i1,i2,i3,i4)
        xw = np.zeros((6, L1_KMAX, B_SH, S_IN, S_IN), np.float32)
        for t, (o3a, T3, o4a, T4) in enumerate(L1_TILES):
            W3, W4 = T3 + 3, T4 + 3
            win = xs[:, :, :, o3a:o3a + W3, o4a:o4a + W4]  # [B,18,18,W3,W4]
            xw[t, :W3 * W4] = win.transpose(3, 4, 0, 1, 2).reshape(W3 * W4, B_SH, S_IN, S_IN)
        shards.append(np.ascontiguousarray(xw).astype(_BF16))
    return consts, fc2b, shards


def _scatter_plan():
    """Per L1-source-tile list of (window, dst_part, src_part, n_part) frags,
    plus per-window readiness (last source tile index)."""
    by_src = {t: [] for t in range(6)}
    ready = {}
    for u, v in product(range(4), range(3)):
        last = 0
        for i3r in range(L2_W3):
            o3 = 3 * u + i3r
            s, o3r = divmod(o3, 5)
            for g0, g1 in ((4 * v, min(4 * v + 7, 8)), (max(4 * v, 8), 4 * v + 7)):
                if g0 >= g1:
                    continue
                tt = 0 if g0 < 8 else 1
                o4a, T4 = (0, 8) if tt == 0 else (8, 7)
                st = s * 2 + tt
                sp = (o3r * T4 + (g0 - o4a)) * 3
                dp = (i3r * L2_W4 + (g0 - 4 * v)) * 3
                by_src[st].append(((u, v), dp, sp, (g1 - g0) * 3))
                last = max(last, st)
        ready[(u, v)] = last
    order = sorted(ready, key=lambda w: (ready[w], w))
    return by_src, order


def _build_module(consts, fc2b, n_chunks=N_CHUNKS_FULL, split_waits=True):
    import concourse.bass as bass
    import concourse.tile as tile
    import concourse.mybir as mybir

    b_sh = n_chunks * B_C
    dt = mybir.dt
    AF = mybir.ActivationFunctionType
    nc = bass.Bass("TRN2", target_bir_lowering=False, debug=False,
                   enable_asserts=False, num_devices=N_CORES)

    x_d = nc.dram_tensor("x", [6, L1_KMAX, b_sh, S_IN, S_IN], dt.bfloat16,
                         kind="ExternalInput").ap()
    out_d = nc.dram_tensor("out", [1, b_sh], dt.float32, kind="ExternalOutput").ap()
    cd = {k: nc.inline_tensor(v, name=k) for k, v in consts.items()}

    with tile.TileContext(nc) as tc:
        with ExitStack() as ctx:
            wpool = ctx.enter_context(tc.tile_pool(name="wts", bufs=1))
            xpool = ctx.enter_context(tc.tile_pool(name="xin", bufs=2))
            apool = ctx.enter_context(tc.tile_pool(name="acts", bufs=1))
            rpool = ctx.enter_context(tc.tile_pool(name="rbuf", bufs=2))
            opool = ctx.enter_context(tc.tile_pool(name="outb", bufs=1))
            pspool = ctx.enter_context(tc.tile_pool(name="ps", bufs=4, space="PSUM"))
            fcps = ctx.enter_context(tc.tile_pool(name="fcps", bufs=1, space="PSUM"))

            # ---- load constants to SBUF ----
            sb = {}
            dmap = {np.dtype(_BF16): dt.bfloat16, np.dtype(_F8): dt.float8e4,
                    np.dtype(np.float32): dt.float32}
            for k, v in consts.items():
                t = wpool.tile(list(v.shape), dmap[v.dtype], tag=k)
                nc.sync.dma_start(t[:], cd[k].ap())
                sb[k] = t

            h_all = apool.tile([60, b_sh, 64], dt.bfloat16)   # (o4,co) x (b, o3o1o2)
            by_src, worder = _scatter_plan()
            dmaq = [nc.sync, nc.gpsimd]          # alternate HWDGE / SWDGE

            for chunk in range(n_chunks):
                b0 = chunk * B_C
                qi = 0
                # ---- input DMA (windowed layout, one per L1 tile) ----
                xw = []
                for t, (o3a, T3, o4a, T4) in enumerate(L1_TILES):
                    Kt = (T3 + 3) * (T4 + 3)
                    xt = xpool.tile([L1_KMAX, B_C, S_IN, S_IN], dt.bfloat16, tag=f"xw{t}")
                    nc.sync.dma_start(xt[0:Kt], x_d[t, 0:Kt, b0:b0 + B_C])
                    xw.append(xt)

                # window tiles allocated up front so scatters can land early
                a2w = {w: apool.tile([L2_K, B_C, 15, 15], dt.bfloat16,
                                     tag=f"a2w{w[0]}_{w[1]}",
                                     name=f"a2w{w[0]}_{w[1]}")
                       for w in product(range(4), range(3))}

                # ---- L1 (+ scatter frags as soon as their source is done) ----
                for t, (o3a, T3, o4a, T4) in enumerate(L1_TILES):
                    Kt, Mt = (T3 + 3) * (T4 + 3), T3 * T4 * 3
                    wt = sb["w1f"] if T4 == 8 else sb["w1e"]
                    bt = sb["b1f"] if T4 == 8 else sb["b1e"]
                    rt = rpool.tile([Mt, B_C, 15, 15], dt.bfloat16, tag=f"r1_{t}")
                    for h in range(2):
                        ps = pspool.tile([Mt, 2, 15, 15], dt.float32, tag="ps")
                        for di, (d1, d2) in enumerate(product(range(4), range(4))):
                            nc.tensor.matmul(
                                ps[:], wt[0:Kt, di, :],
                                xw[t][0:Kt, 2 * h:2 * h + 2, d1:d1 + 15, d2:d2 + 15],
                                start=(di == 0), stop=(di == 15))
                        nc.scalar.activation(rt[:, 2 * h:2 * h + 2], ps[:],
                                             AF.Relu, bias=bt[:])
                    for w, dp, sp, n in by_src[t]:
                        dmaq[qi % 2].dma_start(a2w[w][dp:dp + n], rt[sp:sp + n])
                        qi += 1

                # ---- L2 (windows in scatter-readiness order) ----
                # a3 [54=(i4h,ci), (b, half, flat 12^3 + 64B pad)] fp8: the
                # two i4-halves are the DoubleRow slabs of L3's contraction
                a3 = apool.tile([54, B_C, 2, 1792], dt.float8e4, tag="a3")
                if chunk == 0:
                    nc.sync.dma_start(a3[:, :, :, 1728:1792], sb["a3pad"][:])
                for u, v in worder:
                    rt = rpool.tile([L2_M, B_C, 12, 12], dt.float8e4, tag="r2", bufs=3)
                    for h in range(2):
                        ps = pspool.tile([L2_M, 2, 12, 12], dt.float32, tag="ps")
                        for di, (d1, d2) in enumerate(product(range(4), range(4))):
                            nc.tensor.matmul(
                                ps[:], sb["w2t"][:, di, :],
                                a2w[(u, v)][:, 2 * h:2 * h + 2, d1:d1 + 12, d2:d2 + 12],
                                start=(di == 0), stop=(di == 15))
                        nc.scalar.activation(rt[:, 2 * h:2 * h + 2], ps[:],
                                             AF.Relu, bias=sb["b2t"][:])
                    # scatter -> a3: i4 = 4v + o4r -> half h = i4//6, part
                    # (i4h*9 + ci); contiguous runs split at the h boundary
                    for o3r in range(3):
                        j = (3 * u + o3r) * 144
                        for r0, n, h, p0 in (
                            ((0, 36, 0, 36 * v),) if v == 0 else
                            ((0, 18, 0, 36), (18, 18, 1, 0)) if v == 1 else
                            ((0, 36, 1, 18),)
                        ):
                            dmaq[qi % 2].dma_start(
                                a3[p0:p0 + n, :, h, j:j + 144],
                                rt[36 * o3r + r0:36 * o3r + r0 + n])
                            qi += 1

                # ---- L3 (fp8 DoubleRow over i4-halves; full-width 12x12
                #      columns, valid 9x9 evacuated) ----
                a4 = apool.tile([108, B_C, 9, 9, 9], dt.bfloat16, tag="a4")
                for bi in range(B_C):
                    for c0 in (0, 3, 6):
                        ps = pspool.tile([128, 432], dt.float32, tag="ps")
                        for di, (d1, d2, d3) in enumerate(product(range(4), repeat=3)):
                            off = (d3 + c0) * 144 + d1 * 12 + d2
                            nc.tensor.matmul(
                                ps[:], sb["w3t"][:, di, :, :],
                                a3[:, bi, :, off:off + 432],
                                start=(di == 0), stop=(di == 63),
                                perf_mode=mybir.MatmulPerfMode.DoubleRow)
                        psv = ps[0:108].rearrange("p (c y z) -> p c y z", c=3, y=12)
                        nc.scalar.activation(a4[:, bi, c0:c0 + 3],
                                             psv[:, :, 0:9, 0:9],
                                             AF.Relu, bias=sb["b3t"][:],
                                             scale=1.0 / W3_SCALE)

                # ---- L4 ----
                a5 = apool.tile([90, B_C, 6, 6, 6], dt.bfloat16, tag="a5")
                for bi in range(B_C):
                    ps = pspool.tile([90, 6, 6, 6], dt.float32, tag="ps")
                    for di, (d1, d2, d3) in enumerate(product(range(4), repeat=3)):
                        nc.tensor.matmul(
                            ps[:], sb["w4t"][:, di, :],
                            a4[:, bi, d3:d3 + 6, d1:d1 + 6, d2:d2 + 6],
                            start=(di == 0), stop=(di == 63))
                    nc.scalar.activation(a5[:, bi], ps[:], AF.Relu, bias=sb["b4t"][:])

                # ---- L5 ----
                ps5 = pspool.tile([60, B_C, 4, 4, 4], dt.float32, tag="ps")
                for bi in range(B_C):
                    for di, (d1, d2, d3) in enumerate(product(range(3), repeat=3)):
                        nc.tensor.matmul(
                            ps5[:, bi], sb["w5t"][:, di, :],
                            a5[:, bi, d3:d3 + 4, d1:d1 + 4, d2:d2 + 4],
                            start=(di == 0), stop=(di == 26))
                nc.scalar.activation(
                    h_all[:, b0:b0 + B_C],
                    ps5[:].rearrange("p b x y z -> p b (x y z)"),
                    AF.Relu, bias=sb["b5t"][:])

            # ---- FC head ----
            ps1 = fcps.tile([FC1_N, b_sh], dt.float32, tag="psfc")
            for f in range(64):
                nc.tensor.matmul(ps1[:], sb["fc1w"][:, f, :], h_all[:, :, f],
                                 start=(f == 0), stop=(f == 63))
            r1t = opool.tile([FC1_N, b_sh], dt.bfloat16, tag="rfc")
            nc.scalar.activation(r1t[:], ps1[:], AF.Relu, bias=sb["fc1b"][:])
            ps2 = fcps.tile([1, b_sh], dt.float32, tag="psfc2")
            nc.tensor.matmul(ps2[:], sb["fc2w"][:], r1t[:], start=True, stop=True)
            ob = opool.tile([1, b_sh], dt.float32, tag="ob")
            nc.scalar.activation(ob[:], ps2[:], AF.Sigmoid, bias=fc2b)
            nc.sync.dma_start(out_d, ob[:])

    if split_waits:
        _split_excess_waits(nc)
    return nc


def _split_excess_waits(nc, max_waits=1):
    """Hoist excess sem-waits onto injected wait-only InstEventSemaphore
    carriers immediately before the instruction on the same engine."""
    import concourse.mybir as mybir
    f = nc.m.functions[0]
    ctr = 0
    for blk in f.blocks:
        il = blk.instructions
        i = 0
        while i < len(il):
            inst = il[i]
            si = inst.sync_info
            ty = type(inst).__name__
            lim = 2 if ty == "InstEventSemaphore" else max_waits
            if si is not None and si.on_wait and len(si.on_wait) > lim:
                waits = list(si.on_wait)
                si.on_wait = waits[:lim]
                for w in waits[lim:]:
                    ev = mybir.InstEventSemaphore(name=f"wsplit_{ctr}", ins=[], outs=[])
                    ctr += 1
                    ev.engine = inst.engine
                    ev.sync_info = mybir.SyncInfo(on_wait=[w], on_update=[])
                    il.insert(i, ev)
                    i += 1
            i += 1
    return ctr


def kernel(**inputs) -> np.ndarray:
    import concourse.bass_utils as bass_utils
    consts, fc2b, shards = _host_prep(inputs)
    nc = _build_module(consts, fc2b)
    in_maps = [{"x": s} for s in shards]
    r = bass_utils.run_bass_kernel_spmd(nc, in_maps, core_ids=list(range(N_CORES)))
    outs = [r.results[c]["out"].reshape(B_SH, 1) for c in range(N_CORES)]
    return np.concatenate(outs, axis=0).astype(np.float32)
